# revision 18
# baseline (speedup 1.0000x reference)
import numpy as np

# nn_MyTemporalAttention: x [32, 64, 2048] -> y [32, 2048, 64]
B, C, L = 32, 64, 2048
KQ = 50
NCORES = 8
BPC = B // NCORES          # batches per core
NM = L // 128              # m-chunks of 128

TRACE = False
LAST_EXEC_NS = None
_cache = {}

# --- tuning knobs ---
N_FILL = 50                # dummy matmuls to keep HAM warm during prologue
# 4-chunk groups (global index b*4 + m//4, 16 total) where ACT does BOTH
# exp halves; in the rest the DVE takes h1. 5/16 -> ACT:DVE tiles 84:44.
ACT_BOTH_G = {0, 3, 6, 9, 13}


def _build(scale: float):
    import concourse.bass as bass
    import concourse.tile as tile
    from concourse import bacc, mybir
    from contextlib import ExitStack

    FP32 = mybir.dt.float32
    BF16 = mybir.dt.bfloat16
    I32 = mybir.dt.int32
    I16 = mybir.dt.int16
    AF = mybir.ActivationFunctionType
    OP = mybir.AluOpType
    AX = mybir.AxisListType
    ts = bass.ts

    # Schraudolph exp (fp32/int32 space) for elu (input <= 0)
    SA = float(2**23 / np.log(2))
    SB = float(127 * 2**23 - 486408)
    # Schraudolph exp in bf16/int16 bit space for the softmax exps: valid
    # since |w*scale| < 4 for this problem's data
    SA16 = float(2**7 / np.log(2))
    SB16 = float(127 * 2**7 - 486408.0 / 2**16)

    nc = bacc.Bacc(
        "TRN2",
        target_bir_lowering=False,
        debug=False,
        enable_asserts=False,
        num_devices=NCORES,
    )
    x_d = nc.dram_tensor("x", [BPC, C + 1, L], BF16, kind="ExternalInput").ap()
    wkq_d = nc.dram_tensor("wkq", [C + 1, 128], BF16, kind="ExternalInput").ap()
    wv_d = nc.dram_tensor("wv", [C + 1, C], BF16, kind="ExternalInput").ap()
    # y packed: [BPC, 2, 64, 1024]; [b, h, c, j] = y[b, h*1024+j, c]
    y_d = nc.dram_tensor("y", [BPC, 2, C, 1024], FP32, kind="ExternalOutput").ap()

    with tile.TileContext(nc) as tc, ExitStack() as ctx:
        const = ctx.enter_context(tc.tile_pool(name="const", bufs=1))
        xpool = ctx.enter_context(tc.tile_pool(name="xp", bufs=BPC))
        kqpool = ctx.enter_context(tc.tile_pool(name="kqt", bufs=2))
        kq2pool = ctx.enter_context(tc.tile_pool(name="kq2", bufs=2))
        xmpool = ctx.enter_context(tc.tile_pool(name="xm", bufs=2))
        xepool = ctx.enter_context(tc.tile_pool(name="xe", bufs=2))
        xqpool = ctx.enter_context(tc.tile_pool(name="xq", bufs=4))
        vpool = ctx.enter_context(tc.tile_pool(name="v", bufs=2))
        epool = ctx.enter_context(tc.tile_pool(name="e", bufs=8))
        ytpool = ctx.enter_context(tc.tile_pool(name="yt", bufs=2))
        spool = ctx.enter_context(tc.tile_pool(name="stats", bufs=16))
        vppool = ctx.enter_context(tc.tile_pool(name="vp", bufs=4))
        fpool = ctx.enter_context(tc.tile_pool(name="f1", bufs=3))
        # PSUM: pw 3 x [128,1024] f32 (6 banks) + py 1 x [128,1024] f32 (2)
        pw = ctx.enter_context(tc.tile_pool(name="pw", bufs=3, space="PSUM"))
        py = ctx.enter_context(tc.tile_pool(name="py", bufs=1, space="PSUM"))

        # trigger the exp table-set load early so it overlaps the prologue
        warm = const.tile([1, 2], FP32)
        nc.vector.memset(warm[0:1, 0:1], 0.0)
        nc.scalar.activation(warm[0:1, 1:2], warm[0:1, 0:1], AF.Exp)

        wkq = const.tile([C + 1, 128], BF16)
        nc.sync.dma_start(wkq[:], wkq_d[:])
        xps = []
        for b in range(BPC):
            xp = xpool.tile([C + 1, L], BF16)
            xps.append(xp)
        nc.sync.dma_start(xps[0][:], x_d[0])
        wv = const.tile([C + 1, C], BF16)
        nc.sync.dma_start(wv[:], wv_d[:])
        for b in range(1, BPC):
            nc.sync.dma_start(xps[b][:], x_d[b])

        def prep_tasks(b):
            """Emission closures producing kqt/kq2/vsb for batch b."""
            xp = xps[b]
            # kqt: k rows at partitions 0-49, q rows at partitions 64-113
            # kq2 (mirror): q rows at partitions 0-49, k rows at 64-113
            kqt = kqpool.tile([128, L], BF16)
            kq2 = kq2pool.tile([128, L], BF16)
            vsb = vpool.tile([128, NM * C], BF16)
            pkqs = [None, None]

            def kq_mm(h):
                pkq = pw.tile([128, 1024], FP32, name="pwm")
                pkqs[h] = pkq
                for j in range(2):
                    nc.tensor.matmul(
                        pkq[:, ts(j, 512)], wkq[:], xp[:, ts(2 * h + j, 512)],
                        start=True, stop=True,
                    )

            # elu split into 3 ops so the DVE stream stays fine-grained
            xms = [None, None]
            xes = [None, None]

            def elu_a(h):
                xm = xmpool.tile([128, 1024], BF16)
                xms[h] = xm
                nc.vector.tensor_scalar_min(xm[:], pkqs[h][:], 0.0)

            def elu_b(h):
                xe = xepool.tile([128, 1024], I32)
                xes[h] = xe
                nc.vector.tensor_scalar(xe[:], xms[h][:], SA, SB, OP.mult, OP.add)

            def elu_c(h):
                nc.vector.scalar_tensor_tensor(
                    kqt[:, ts(h, 1024)], xes[h][:].bitcast(FP32), -1.0,
                    pkqs[h][:], OP.add, OP.max,
                )

            def qdup():
                nc.sync.dma_start(kq2[0:KQ, :], kqt[64 : 64 + KQ, :])
                nc.sync.dma_start(kq2[64 : 64 + KQ, :], kqt[0:KQ, :])

            def qdup_half(h):
                sl = slice(1024 * h, 1024 * (h + 1))
                nc.sync.dma_start(kq2[0:KQ, sl], kqt[64 : 64 + KQ, sl])
                nc.sync.dma_start(kq2[64 : 64 + KQ, sl], kqt[0:KQ, sl])

            pvs = [None]

            def v_mm(part):
                if part == 0:
                    pvs[0] = pw.tile([128, 1024], FP32, name="pwm")
                pv = pvs[0]
                for jj in range(8 * part, 8 * part + 8):
                    nc.tensor.matmul(
                        pv[:, ts(jj, C)], xp[:, ts(jj, 128)], wv[:],
                        start=True, stop=True,
                    )

            def v_tanh():
                nc.scalar.activation(vsb[:], pvs[0][:], AF.Tanh)

            if b == 0:
                tasks = [
                    lambda: kq_mm(0),
                    lambda: (elu_a(0), elu_b(0), elu_c(0)),
                    lambda: qdup_half(0),
                    lambda: kq_mm(1),
                    lambda: (elu_a(1), elu_b(1), elu_c(1)),
                    lambda: qdup_half(1),
                    lambda: v_mm(0),
                    lambda: v_mm(1),
                    v_tanh,
                ]
            else:
                tasks = []
                for h in range(2):
                    tasks.append(lambda h=h: kq_mm(h))
                    tasks.append(lambda h=h: elu_a(h))
                    tasks.append(lambda h=h: elu_b(h))
                    tasks.append(lambda h=h: elu_c(h))
                tasks.append(qdup)
                tasks.append(lambda: v_mm(0))
                tasks.append(lambda: v_mm(1))
                tasks.append(v_tanh)
            return kqt, kq2, vsb, tasks

        # PE warmup: back-to-back dummy matmuls during the DMA prologue
        # to engage the HAM 8/8 clock
        wz = const.tile([128, 512], BF16)
        nc.vector.memset(wz[:], 0.0)
        # warm the DVE accum-reduce uop path (first use gives a bad sum)
        wd = const.tile([128, 1], FP32)
        nc.vector.tensor_scalar(
            wz[:, 0:64], wz[:, 0:64], 1.0, 0.0, OP.mult, OP.add,
            accum_out=wd[:],
        )
        pwarm = pw.tile([128, 1024], FP32, name="pwm")
        for r in range(10):
            nc.tensor.matmul(
                pwarm[:, 0:512], wz[:, 0:128], wz[:],
                start=True, stop=True,
            )

        kqt, kq2, vsb, tasks0 = prep_tasks(0)
        for t in tasks0[:6]:
            t()

        # dummy matmuls to keep the PE busy (HAM stays at 8/8) while the
        # DVE computes batch-0 elu; the real mm2 stream follows in-order
        dumt = pw.tile([128, 1024], FP32, name="pwm")
        for r in range(N_FILL):
            nc.tensor.matmul(
                dumt[:, ts(r % 2, 512)], wz[:, 0:128], wz[:],
                start=True, stop=True,
            )

        for b in range(BPC):
            if b + 1 < BPC:
                kqt_n, kq2_n, vsb_n, tasks = prep_tasks(b + 1)
            else:
                tasks = []

            pyt = py.tile([128, 1024], FP32, name="pyt")

            def emit_mm2(m):
                # Two row-tiled streams: A on PE rows 0-49 (l-half 0),
                # B on rows 64-113 (l-half 1).
                tiles = [
                    pw.tile([128, 1024], FP32, name="pwm"),
                    pw.tile([128, 1024], FP32, name="pwm"),
                ]
                for jj in range(2):
                    nc.tensor.matmul(
                        tiles[0][:, ts(jj, 512)],
                        kq2[0:KQ, ts(m, 128)],
                        kqt[0:KQ, ts(jj, 512)],
                        start=True,
                        stop=True,
                    )
                    nc.tensor.matmul(
                        tiles[1][:, ts(jj, 512)],
                        kqt[64 : 64 + KQ, ts(m, 128)],
                        kq2[64 : 64 + KQ, ts(2 + jj, 512)],
                        start=True,
                        stop=True,
                    )
                return tiles

            pw2 = emit_mm2(0)
            if b == 0:
                for t in tasks0[6:]:
                    t()
            ti = 0

            def emit_mm3(m, vp, eh0_ap, eh1_ap):
                # y^T accum: quadrant selects partition half; order
                # 0,2,1,3 pairs different col-groups for concurrency
                for j in (0, 2, 1, 3):
                    eh = eh0_ap if j < 2 else eh1_ap
                    nc.tensor.matmul(
                        pyt[64 * (j // 2) : 64 * (j // 2) + C, ts(j % 2, 512)],
                        vp[:],
                        eh[:, ts(j % 2, 512)],
                        start=(m == 0),
                        stop=(m == NM - 1),
                    )

            # per-group state (groups of 4 m-chunks)
            ehs = [None] * 8
            sgA = None
            sgH = None

            for m in range(NM):
                mi = m % 4
                if mi == 0:
                    sgA = spool.tile([128, 4], FP32)
                    sgH = spool.tile([128, 4], FP32)
                eh0 = epool.tile([128, 1024], BF16)
                ehs[2 * mi] = eh0[:]
                # h0 on ACT (native exp, free row-sum via accumulator)
                nc.scalar.activation(
                    eh0[:], pw2[0][:], AF.Exp, scale=scale,
                    accum_out=sgA[:, mi : mi + 1],
                )
                if (b * 4 + m // 4) in ACT_BOTH_G:
                    eh1 = epool.tile([128, 1024], BF16)
                    nc.scalar.activation(
                        eh1[:], pw2[1][:], AF.Exp, scale=scale,
                        accum_out=sgH[:, mi : mi + 1],
                    )
                    ehs[2 * mi + 1] = eh1[:]
                else:
                    # h1 on DVE: Schraudolph exp in bf16 bit space; the
                    # int16 bits ARE the bf16 et values, so mm3 reads the
                    # bitcast tile and only the row-sum needs a second op
                    xq = xqpool.tile([128, 1024], I16)
                    nc.vector.tensor_scalar(
                        xq[:], pw2[1][:], SA16 * scale, SB16, OP.mult, OP.add,
                    )
                    eh1_ap = xq[:].bitcast(BF16)
                    nc.vector.tensor_reduce(
                        sgH[:, mi : mi + 1], eh1_ap, AX.X, OP.add
                    )
                    ehs[2 * mi + 1] = eh1_ap
                if m + 1 < NM:
                    pw2 = emit_mm2(m + 1)
                # zero-weight accumulate into pyt: keeps the PE stream dense
                # through the stats/exp waits so HAM stays at K=8/8
                nc.tensor.matmul(
                    pyt[:, 0:512], wz[0:KQ, 0:128], wz[0:KQ, :],
                    start=False, stop=False, skip_group_check=True,
                )
                if mi == 3:
                    # batched softmax denominators for the 4-chunk group
                    d4 = spool.tile([128, 4], FP32)
                    nc.vector.tensor_add(d4[:], sgA[:], sgH[:])
                    dv4 = spool.tile([128, 4], FP32)
                    nc.vector.reciprocal(dv4[:], d4[:])
                    vpg = vppool.tile([128, 256], BF16)
                    g0 = m - 3
                    in0 = vsb[:, g0 * C : (m + 1) * C].rearrange(
                        "p (c k) -> p c k", c=4
                    )
                    in1 = dv4[:].unsqueeze(2)
                    i0b, i1b = bass.broadcast_tensor_aps(in0, in1)
                    nc.vector.tensor_mul(
                        vpg[:].rearrange("p (c k) -> p c k", c=4), i0b, i1b
                    )
                    for k in range(4):
                        emit_mm3(
                            g0 + k, vpg[:, ts(k, C)],
                            ehs[2 * k], ehs[2 * k + 1],
                        )
                if ti < len(tasks) and m >= 2:
                    tasks[ti]()
                    ti += 1

            while ti < len(tasks):
                tasks[ti]()
                ti += 1

            yt = ytpool.tile([128, 1024], FP32)
            nc.scalar.copy(yt[:], pyt[:])
            nc.sync.dma_start(y_d[b][0], yt[0:C, :])
            nc.sync.dma_start(y_d[b][1], yt[64 : 64 + C, :])

            if b + 1 < BPC:
                kqt, kq2, vsb = kqt_n, kq2_n, vsb_n

    nc.finalize()
    return nc


def kernel(x, Wk, bk, Wq, bq, Wv, bv, sample_len):
    global LAST_EXEC_NS
    from concourse.bass_utils import run_bass_kernel_spmd

    scale = float(1.0 / np.sqrt(np.float64(sample_len)))
    if scale not in _cache:
        _cache[scale] = _build(scale)
    nc = _cache[scale]

    import ml_dtypes

    bf16 = ml_dtypes.bfloat16
    x = np.asarray(x, dtype=np.float32)
    ones = np.ones((B, 1, L), dtype=np.float32)
    x = np.ascontiguousarray(np.concatenate([x, ones], axis=1)).astype(bf16)
    wkq = np.zeros((C + 1, 128), dtype=np.float32)
    wkq[:, 0:KQ] = np.concatenate([Wk, bk[None, :]], axis=0)
    wkq[:, 64 : 64 + KQ] = np.concatenate([Wq, bq[None, :]], axis=0)
    wkq = wkq.astype(bf16)
    wv = np.concatenate([Wv, bv[None, :]], axis=0).astype(bf16)

    in_maps = [
        {"x": x[i * BPC : (i + 1) * BPC], "wkq": wkq, "wv": wv}
        for i in range(NCORES)
    ]
    res = run_bass_kernel_spmd(nc, in_maps, list(range(NCORES)), trace=TRACE)
    LAST_EXEC_NS = res.exec_time_ns
    yp = np.concatenate([res.results[i]["y"] for i in range(NCORES)], axis=0)
    # yp: [B, 2, 64, 1024] -> y: [B, 2048, 64]
    y = yp.transpose(0, 1, 3, 2).reshape(B, L, C)
    return np.ascontiguousarray(y)


# revision 23
# speedup vs baseline: 1.1977x; 1.1977x over previous
import numpy as np

# nn_MyTemporalAttention: x [32, 64, 2048] -> y [32, 2048, 64]
B, C, L = 32, 64, 2048
KQ = 50
NCORES = 8
BPC = B // NCORES          # batches per core
NM = L // 128              # m-chunks of 128

TRACE = False
LAST_EXEC_NS = None
_cache = {}

# --- tuning knobs ---
N_FILL = 50                # dummy matmuls to keep HAM warm during prologue
# 4-chunk groups (global index b*4 + m//4, 16 total) where ACT does BOTH
# exp halves; in the rest the DVE takes h1. 5/16 -> ACT:DVE tiles 84:44.
ACT_BOTH_G = {0, 3, 6, 9, 13}


def _build(scale: float):
    import concourse.bass as bass
    import concourse.tile as tile
    from concourse import bacc, mybir
    from contextlib import ExitStack

    FP32 = mybir.dt.float32
    BF16 = mybir.dt.bfloat16
    I32 = mybir.dt.int32
    I16 = mybir.dt.int16
    AF = mybir.ActivationFunctionType
    OP = mybir.AluOpType
    AX = mybir.AxisListType
    ts = bass.ts

    # Schraudolph exp (fp32/int32 space) for elu (input <= 0)
    SA = float(2**23 / np.log(2))
    SB = float(127 * 2**23 - 486408)
    # Schraudolph exp in bf16/int16 bit space for the softmax exps: valid
    # since |w*scale| < 4 for this problem's data
    SA16 = float(2**7 / np.log(2))
    SB16 = float(127 * 2**7 - 486408.0 / 2**16)

    nc = bacc.Bacc(
        "TRN2",
        target_bir_lowering=False,
        debug=False,
        enable_asserts=False,
        num_devices=NCORES,
    )
    x_d = nc.dram_tensor("x", [BPC, C + 1, L], BF16, kind="ExternalInput").ap()
    wkq_d = nc.dram_tensor("wkq", [C + 1, 128], BF16, kind="ExternalInput").ap()
    wv_d = nc.dram_tensor("wv", [C + 1, C], BF16, kind="ExternalInput").ap()
    # y packed: [BPC, 2, 64, 1024]; [b, h, c, j] = y[b, h*1024+j, c]
    y_d = nc.dram_tensor("y", [BPC, 2, C, 1024], FP32, kind="ExternalOutput").ap()

    with tile.TileContext(nc) as tc, ExitStack() as ctx:
        const = ctx.enter_context(tc.tile_pool(name="const", bufs=1))
        xpool = ctx.enter_context(tc.tile_pool(name="xp", bufs=BPC))
        kqpool = ctx.enter_context(tc.tile_pool(name="kqt", bufs=2))
        kq2pool = ctx.enter_context(tc.tile_pool(name="kq2", bufs=2))
        xmpool = ctx.enter_context(tc.tile_pool(name="xm", bufs=2))
        xepool = ctx.enter_context(tc.tile_pool(name="xe", bufs=2))
        xqpool = ctx.enter_context(tc.tile_pool(name="xq", bufs=7))
        vpool = ctx.enter_context(tc.tile_pool(name="v", bufs=2))
        epool = ctx.enter_context(tc.tile_pool(name="e", bufs=16))
        ytpool = ctx.enter_context(tc.tile_pool(name="yt", bufs=2))
        spool = ctx.enter_context(tc.tile_pool(name="stats", bufs=16))
        vppool = ctx.enter_context(tc.tile_pool(name="vp", bufs=4))
        fpool = ctx.enter_context(tc.tile_pool(name="f1", bufs=3))
        # PSUM: pw 3 x [128,1024] f32 (6 banks) + py 1 x [128,1024] f32 (2)
        pw = ctx.enter_context(tc.tile_pool(name="pw", bufs=3, space="PSUM"))
        py = ctx.enter_context(tc.tile_pool(name="py", bufs=1, space="PSUM"))

        # trigger the exp table-set load early so it overlaps the prologue
        warm = const.tile([1, 2], FP32)
        nc.vector.memset(warm[0:1, 0:1], 0.0)
        nc.scalar.activation(warm[0:1, 1:2], warm[0:1, 0:1], AF.Exp)

        wkq = const.tile([C + 1, 128], BF16)
        nc.sync.dma_start(wkq[:], wkq_d[:])
        xps = []
        for b in range(BPC):
            xp = xpool.tile([C + 1, L], BF16)
            xps.append(xp)
        nc.sync.dma_start(xps[0][:], x_d[0])
        wv = const.tile([C + 1, C], BF16)
        nc.sync.dma_start(wv[:], wv_d[:])
        for b in range(1, BPC):
            nc.sync.dma_start(xps[b][:], x_d[b])

        def prep_tasks(b):
            """Emission closures producing kqt/kq2/vsb for batch b."""
            xp = xps[b]
            # kqt: k rows at partitions 0-49, q rows at partitions 64-113
            # kq2 (mirror): q rows at partitions 0-49, k rows at 64-113
            kqt = kqpool.tile([128, L], BF16)
            kq2 = kq2pool.tile([128, L], BF16)
            vsb = vpool.tile([128, NM * C], BF16)
            pkqs = [None, None]

            def kq_mm(h):
                pkq = pw.tile([128, 1024], FP32, name="pwm")
                pkqs[h] = pkq
                for j in range(2):
                    nc.tensor.matmul(
                        pkq[:, ts(j, 512)], wkq[:], xp[:, ts(2 * h + j, 512)],
                        start=True, stop=True,
                    )

            # elu split into 3 ops so the DVE stream stays fine-grained
            xms = [None, None]
            xes = [None, None]

            def elu_a(h):
                xm = xmpool.tile([128, 1024], BF16)
                xms[h] = xm
                nc.vector.tensor_scalar_min(xm[:], pkqs[h][:], 0.0)

            def elu_b(h):
                xe = xepool.tile([128, 1024], I32)
                xes[h] = xe
                nc.vector.tensor_scalar(xe[:], xms[h][:], SA, SB, OP.mult, OP.add)

            def elu_c(h):
                nc.vector.scalar_tensor_tensor(
                    kqt[:, ts(h, 1024)], xes[h][:].bitcast(FP32), -1.0,
                    pkqs[h][:], OP.add, OP.max,
                )

            def qdup():
                nc.sync.dma_start(kq2[0:KQ, :], kqt[64 : 64 + KQ, :])
                nc.sync.dma_start(kq2[64 : 64 + KQ, :], kqt[0:KQ, :])

            def qdup_half(h):
                sl = slice(1024 * h, 1024 * (h + 1))
                nc.sync.dma_start(kq2[0:KQ, sl], kqt[64 : 64 + KQ, sl])
                nc.sync.dma_start(kq2[64 : 64 + KQ, sl], kqt[0:KQ, sl])

            pvs = [None]

            def v_mm(part):
                if part == 0:
                    pvs[0] = pw.tile([128, 1024], FP32, name="pwm")
                pv = pvs[0]
                for jj in range(8 * part, 8 * part + 8):
                    nc.tensor.matmul(
                        pv[:, ts(jj, C)], xp[:, ts(jj, 128)], wv[:],
                        start=True, stop=True,
                    )

            def v_tanh():
                nc.scalar.activation(vsb[:], pvs[0][:], AF.Tanh)

            if b == 0:
                tasks = [
                    lambda: kq_mm(0),
                    lambda: (elu_a(0), elu_b(0), elu_c(0)),
                    lambda: qdup_half(0),
                    lambda: kq_mm(1),
                    lambda: (elu_a(1), elu_b(1), elu_c(1)),
                    lambda: qdup_half(1),
                    lambda: v_mm(0),
                    lambda: v_mm(1),
                    v_tanh,
                ]
            else:
                tasks = []
                for h in range(2):
                    tasks.append(lambda h=h: kq_mm(h))
                    tasks.append(lambda h=h: elu_a(h))
                    tasks.append(lambda h=h: elu_b(h))
                    tasks.append(lambda h=h: elu_c(h))
                tasks.append(qdup)
                tasks.append(lambda: v_mm(0))
                tasks.append(lambda: v_mm(1))
                tasks.append(v_tanh)
            return kqt, kq2, vsb, tasks

        # PE warmup: back-to-back dummy matmuls during the DMA prologue
        # to engage the HAM 8/8 clock
        wz = const.tile([128, 512], BF16)
        nc.vector.memset(wz[:], 0.0)
        # warm the DVE accum-reduce uop path (first use gives a bad sum)
        wd = const.tile([128, 1], FP32)
        nc.vector.tensor_scalar(
            wz[:, 0:64], wz[:, 0:64], 1.0, 0.0, OP.mult, OP.add,
            accum_out=wd[:],
        )
        pwarm = pw.tile([128, 1024], FP32, name="pwm")
        for r in range(10):
            nc.tensor.matmul(
                pwarm[:, 0:512], wz[:, 0:128], wz[:],
                start=True, stop=True,
            )

        kqt, kq2, vsb, tasks0 = prep_tasks(0)
        for t in tasks0[:6]:
            t()

        # dummy matmuls to keep the PE busy (HAM stays at 8/8) while the
        # DVE computes batch-0 elu; the real mm2 stream follows in-order
        dumt = pw.tile([128, 1024], FP32, name="pwm")
        for r in range(N_FILL):
            nc.tensor.matmul(
                dumt[:, ts(r % 2, 512)], wz[:, 0:128], wz[:],
                start=True, stop=True,
            )

        for b in range(BPC):
            if b + 1 < BPC:
                kqt_n, kq2_n, vsb_n, tasks = prep_tasks(b + 1)
            else:
                tasks = []

            pyt = py.tile([128, 1024], FP32, name="pyt")

            def emit_mm2(m):
                # Two row-tiled streams: A on PE rows 0-49 (l-half 0),
                # B on rows 64-113 (l-half 1).
                tiles = [
                    pw.tile([128, 1024], FP32, name="pwm"),
                    pw.tile([128, 1024], FP32, name="pwm"),
                ]
                for jj in range(2):
                    nc.tensor.matmul(
                        tiles[0][:, ts(jj, 512)],
                        kq2[0:KQ, ts(m, 128)],
                        kqt[0:KQ, ts(jj, 512)],
                        start=True,
                        stop=True,
                    )
                    nc.tensor.matmul(
                        tiles[1][:, ts(jj, 512)],
                        kqt[64 : 64 + KQ, ts(m, 128)],
                        kq2[64 : 64 + KQ, ts(2 + jj, 512)],
                        start=True,
                        stop=True,
                    )
                return tiles

            pw2 = emit_mm2(0)
            if b == 0:
                for t in tasks0[6:]:
                    t()
            ti = 0

            def emit_mm3(m, vp, eh0_ap, eh1_ap):
                # y^T accum: quadrant selects partition half; order
                # 0,2,1,3 pairs different col-groups for concurrency
                for j in (0, 2, 1, 3):
                    eh = eh0_ap if j < 2 else eh1_ap
                    nc.tensor.matmul(
                        pyt[64 * (j // 2) : 64 * (j // 2) + C, ts(j % 2, 512)],
                        vp[:],
                        eh[:, ts(j % 2, 512)],
                        start=(m == 0),
                        stop=(m == NM - 1),
                    )

            # per-group state (groups of 4 m-chunks)
            ehs = [None] * 8
            sgA = None
            sgH = None
            mm3q = []

            for m in range(NM):
                mi = m % 4
                if mi == 0:
                    sgA = spool.tile([128, 4], FP32)
                    sgH = spool.tile([128, 4], FP32)
                eh0 = epool.tile([128, 1024], BF16)
                ehs[2 * mi] = eh0[:]
                # h0 on ACT (native exp, free row-sum via accumulator)
                nc.scalar.activation(
                    eh0[:], pw2[0][:], AF.Exp, scale=scale,
                    accum_out=sgA[:, mi : mi + 1],
                )
                if (b * 4 + m // 4) in ACT_BOTH_G:
                    eh1 = epool.tile([128, 1024], BF16)
                    nc.scalar.activation(
                        eh1[:], pw2[1][:], AF.Exp, scale=scale,
                        accum_out=sgH[:, mi : mi + 1],
                    )
                    ehs[2 * mi + 1] = eh1[:]
                else:
                    # h1 on DVE: Schraudolph exp in bf16 bit space; the
                    # int16 bits ARE the bf16 et values, so mm3 reads the
                    # bitcast tile and only the row-sum needs a second op
                    xq = xqpool.tile([128, 1024], I16)
                    nc.vector.tensor_scalar(
                        xq[:], pw2[1][:], SA16 * scale, SB16, OP.mult, OP.add,
                    )
                    eh1_ap = xq[:].bitcast(BF16)
                    nc.vector.tensor_reduce(
                        sgH[:, mi : mi + 1], eh1_ap, AX.X, OP.add
                    )
                    ehs[2 * mi + 1] = eh1_ap
                if m + 1 < NM:
                    pw2 = emit_mm2(m + 1)
                if mm3q:
                    # one deferred mm3 per chunk keeps the PE stream dense
                    mm3q.pop(0)()
                if mi == 3:
                    # batched softmax denominators for the 4-chunk group
                    d4 = spool.tile([128, 4], FP32)
                    nc.vector.tensor_add(d4[:], sgA[:], sgH[:])
                    dv4 = spool.tile([128, 4], FP32)
                    nc.vector.reciprocal(dv4[:], d4[:])
                    vpg = vppool.tile([128, 256], BF16)
                    g0 = m - 3
                    in0 = vsb[:, g0 * C : (m + 1) * C].rearrange(
                        "p (c k) -> p c k", c=4
                    )
                    in1 = dv4[:].unsqueeze(2)
                    i0b, i1b = bass.broadcast_tensor_aps(in0, in1)
                    nc.vector.tensor_mul(
                        vpg[:].rearrange("p (c k) -> p c k", c=4), i0b, i1b
                    )
                    for k in range(4):
                        mm3q.append(
                            lambda mk=g0 + k, vq=vpg, kk=k,
                            e0=ehs[2 * k], e1=ehs[2 * k + 1]:
                            emit_mm3(mk, vq[:, ts(kk, C)], e0, e1)
                        )
                if ti < len(tasks) and m >= 2:
                    tasks[ti]()
                    ti += 1

            while mm3q:
                mm3q.pop(0)()
            while ti < len(tasks):
                tasks[ti]()
                ti += 1

            yt = ytpool.tile([128, 1024], FP32)
            nc.scalar.copy(yt[:], pyt[:])
            nc.sync.dma_start(y_d[b][0], yt[0:C, :])
            nc.sync.dma_start(y_d[b][1], yt[64 : 64 + C, :])

            if b + 1 < BPC:
                kqt, kq2, vsb = kqt_n, kq2_n, vsb_n

    nc.finalize()
    return nc


def kernel(x, Wk, bk, Wq, bq, Wv, bv, sample_len):
    global LAST_EXEC_NS
    from concourse.bass_utils import run_bass_kernel_spmd

    scale = float(1.0 / np.sqrt(np.float64(sample_len)))
    if scale not in _cache:
        _cache[scale] = _build(scale)
    nc = _cache[scale]

    import ml_dtypes

    bf16 = ml_dtypes.bfloat16
    x = np.asarray(x, dtype=np.float32)
    ones = np.ones((B, 1, L), dtype=np.float32)
    x = np.ascontiguousarray(np.concatenate([x, ones], axis=1)).astype(bf16)
    wkq = np.zeros((C + 1, 128), dtype=np.float32)
    wkq[:, 0:KQ] = np.concatenate([Wk, bk[None, :]], axis=0)
    wkq[:, 64 : 64 + KQ] = np.concatenate([Wq, bq[None, :]], axis=0)
    wkq = wkq.astype(bf16)
    wv = np.concatenate([Wv, bv[None, :]], axis=0).astype(bf16)

    in_maps = [
        {"x": x[i * BPC : (i + 1) * BPC], "wkq": wkq, "wv": wv}
        for i in range(NCORES)
    ]
    res = run_bass_kernel_spmd(nc, in_maps, list(range(NCORES)), trace=TRACE)
    LAST_EXEC_NS = res.exec_time_ns
    yp = np.concatenate([res.results[i]["y"] for i in range(NCORES)], axis=0)
    # yp: [B, 2, 64, 1024] -> y: [B, 2048, 64]
    y = yp.transpose(0, 1, 3, 2).reshape(B, L, C)
    return np.ascontiguousarray(y)


# revision 25
# speedup vs baseline: 1.2804x; 1.0691x over previous
import numpy as np

# nn_MyTemporalAttention: x [32, 64, 2048] -> y [32, 2048, 64]
B, C, L = 32, 64, 2048
KQ = 50
NCORES = 8
BPC = B // NCORES          # batches per core
NM = L // 128              # m-chunks of 128

TRACE = False
LAST_EXEC_NS = None
_cache = {}

# --- tuning knobs ---
N_FILL = 50                # dummy matmuls to keep HAM warm during prologue
# global chunk indices (b*16+m, 64 total) where the DVE computes the h1
# exp half (Schraudolph); the rest run both halves on ACT. 28/64 keeps the
# DVE comfortably below ACT so the per-chunk pipeline stays dense.
_SD = 28
DVE_M = {i for i in range(64) if (i * _SD) // 64 != ((i - 1) * _SD) // 64}


def _build(scale: float):
    import concourse.bass as bass
    import concourse.tile as tile
    from concourse import bacc, mybir
    from contextlib import ExitStack

    FP32 = mybir.dt.float32
    BF16 = mybir.dt.bfloat16
    I32 = mybir.dt.int32
    I16 = mybir.dt.int16
    AF = mybir.ActivationFunctionType
    OP = mybir.AluOpType
    AX = mybir.AxisListType
    ts = bass.ts

    # Schraudolph exp (fp32/int32 space) for elu (input <= 0)
    SA = float(2**23 / np.log(2))
    SB = float(127 * 2**23 - 486408)
    # Schraudolph exp in bf16/int16 bit space for the softmax exps: valid
    # since |w*scale| < 4 for this problem's data
    SA16 = float(2**7 / np.log(2))
    SB16 = float(127 * 2**7 - 486408.0 / 2**16)

    nc = bacc.Bacc(
        "TRN2",
        target_bir_lowering=False,
        debug=False,
        enable_asserts=False,
        num_devices=NCORES,
    )
    x_d = nc.dram_tensor("x", [BPC, C + 1, L], BF16, kind="ExternalInput").ap()
    wkq_d = nc.dram_tensor("wkq", [C + 1, 128], BF16, kind="ExternalInput").ap()
    wv_d = nc.dram_tensor("wv", [C + 1, C], BF16, kind="ExternalInput").ap()
    # y packed: [BPC, 2, 64, 1024]; [b, h, c, j] = y[b, h*1024+j, c]
    y_d = nc.dram_tensor("y", [BPC, 2, C, 1024], FP32, kind="ExternalOutput").ap()

    with tile.TileContext(nc) as tc, ExitStack() as ctx:
        const = ctx.enter_context(tc.tile_pool(name="const", bufs=1))
        xpool = ctx.enter_context(tc.tile_pool(name="xp", bufs=BPC))
        kqpool = ctx.enter_context(tc.tile_pool(name="kqt", bufs=2))
        kq2pool = ctx.enter_context(tc.tile_pool(name="kq2", bufs=2))
        xmpool = ctx.enter_context(tc.tile_pool(name="xm", bufs=2))
        xepool = ctx.enter_context(tc.tile_pool(name="xe", bufs=2))
        xqpool = ctx.enter_context(tc.tile_pool(name="xq", bufs=7))
        vpool = ctx.enter_context(tc.tile_pool(name="v", bufs=2))
        epool = ctx.enter_context(tc.tile_pool(name="e", bufs=16))
        ytpool = ctx.enter_context(tc.tile_pool(name="yt", bufs=2))
        spool = ctx.enter_context(tc.tile_pool(name="stats", bufs=16))
        vppool = ctx.enter_context(tc.tile_pool(name="vp", bufs=4))
        fpool = ctx.enter_context(tc.tile_pool(name="f1", bufs=3))
        # PSUM: pw 3 x [128,1024] f32 (6 banks) + py 1 x [128,1024] f32 (2)
        pw = ctx.enter_context(tc.tile_pool(name="pw", bufs=3, space="PSUM"))
        py = ctx.enter_context(tc.tile_pool(name="py", bufs=1, space="PSUM"))

        # trigger the exp table-set load early so it overlaps the prologue
        warm = const.tile([1, 2], FP32)
        nc.vector.memset(warm[0:1, 0:1], 0.0)
        nc.scalar.activation(warm[0:1, 1:2], warm[0:1, 0:1], AF.Exp)

        wkq = const.tile([C + 1, 128], BF16)
        nc.sync.dma_start(wkq[:], wkq_d[:])
        xps = []
        for b in range(BPC):
            xp = xpool.tile([C + 1, L], BF16)
            xps.append(xp)
        nc.sync.dma_start(xps[0][:], x_d[0])
        wv = const.tile([C + 1, C], BF16)
        nc.sync.dma_start(wv[:], wv_d[:])
        for b in range(1, BPC):
            nc.sync.dma_start(xps[b][:], x_d[b])

        def prep_tasks(b):
            """Emission closures producing kqt/kq2/vsb for batch b."""
            xp = xps[b]
            # kqt: k rows at partitions 0-49, q rows at partitions 64-113
            # kq2 (mirror): q rows at partitions 0-49, k rows at 64-113
            kqt = kqpool.tile([128, L], BF16)
            kq2 = kq2pool.tile([128, L], BF16)
            vsb = vpool.tile([128, NM * C], BF16)
            pkqs = [None, None]

            def kq_mm(h):
                pkq = pw.tile([128, 1024], FP32, name="pwm")
                pkqs[h] = pkq
                for j in range(2):
                    nc.tensor.matmul(
                        pkq[:, ts(j, 512)], wkq[:], xp[:, ts(2 * h + j, 512)],
                        start=True, stop=True,
                    )

            # elu split into 3 ops so the DVE stream stays fine-grained
            xms = [None, None]
            xes = [None, None]

            def elu_a(h):
                xm = xmpool.tile([128, 1024], BF16)
                xms[h] = xm
                nc.vector.tensor_scalar_min(xm[:], pkqs[h][:], 0.0)

            def elu_b(h):
                xe = xepool.tile([128, 1024], I32)
                xes[h] = xe
                nc.vector.tensor_scalar(xe[:], xms[h][:], SA, SB, OP.mult, OP.add)

            def elu_c(h):
                nc.vector.scalar_tensor_tensor(
                    kqt[:, ts(h, 1024)], xes[h][:].bitcast(FP32), -1.0,
                    pkqs[h][:], OP.add, OP.max,
                )

            def qdup():
                nc.sync.dma_start(kq2[0:KQ, :], kqt[64 : 64 + KQ, :])
                nc.sync.dma_start(kq2[64 : 64 + KQ, :], kqt[0:KQ, :])

            def qdup_half(h):
                sl = slice(1024 * h, 1024 * (h + 1))
                nc.sync.dma_start(kq2[0:KQ, sl], kqt[64 : 64 + KQ, sl])
                nc.sync.dma_start(kq2[64 : 64 + KQ, sl], kqt[0:KQ, sl])

            pvs = [None]

            def v_mm(part):
                if part == 0:
                    pvs[0] = pw.tile([128, 1024], FP32, name="pwm")
                pv = pvs[0]
                for jj in range(8 * part, 8 * part + 8):
                    nc.tensor.matmul(
                        pv[:, ts(jj, C)], xp[:, ts(jj, 128)], wv[:],
                        start=True, stop=True,
                    )

            def v_tanh():
                nc.scalar.activation(vsb[:], pvs[0][:], AF.Tanh)

            if b == 0:
                tasks = [
                    lambda: kq_mm(0),
                    lambda: (elu_a(0), elu_b(0), elu_c(0)),
                    lambda: qdup_half(0),
                    lambda: kq_mm(1),
                    lambda: (elu_a(1), elu_b(1), elu_c(1)),
                    lambda: qdup_half(1),
                    lambda: v_mm(0),
                    lambda: v_mm(1),
                    v_tanh,
                ]
            else:
                tasks = []
                for h in range(2):
                    tasks.append(lambda h=h: kq_mm(h))
                    tasks.append(lambda h=h: elu_a(h))
                    tasks.append(lambda h=h: elu_b(h))
                    tasks.append(lambda h=h: elu_c(h))
                tasks.append(qdup)
                tasks.append(lambda: v_mm(0))
                tasks.append(lambda: v_mm(1))
                tasks.append(v_tanh)
            return kqt, kq2, vsb, tasks

        # PE warmup: back-to-back dummy matmuls during the DMA prologue
        # to engage the HAM 8/8 clock
        wz = const.tile([128, 512], BF16)
        nc.vector.memset(wz[:], 0.0)
        # warm the DVE accum-reduce uop path (first use gives a bad sum)
        wd = const.tile([128, 1], FP32)
        nc.vector.tensor_scalar(
            wz[:, 0:64], wz[:, 0:64], 1.0, 0.0, OP.mult, OP.add,
            accum_out=wd[:],
        )
        pwarm = pw.tile([128, 1024], FP32, name="pwm")
        for r in range(10):
            nc.tensor.matmul(
                pwarm[:, 0:512], wz[:, 0:128], wz[:],
                start=True, stop=True,
            )

        kqt, kq2, vsb, tasks0 = prep_tasks(0)
        for t in tasks0[:6]:
            t()

        # dummy matmuls to keep the PE busy (HAM stays at 8/8) while the
        # DVE computes batch-0 elu; the real mm2 stream follows in-order
        dumt = pw.tile([128, 1024], FP32, name="pwm")
        for r in range(N_FILL):
            nc.tensor.matmul(
                dumt[:, ts(r % 2, 512)], wz[:, 0:128], wz[:],
                start=True, stop=True,
            )

        for b in range(BPC):
            if b + 1 < BPC:
                kqt_n, kq2_n, vsb_n, tasks = prep_tasks(b + 1)
            else:
                tasks = []

            pyt = py.tile([128, 1024], FP32, name="pyt")

            def emit_mm2(m):
                # Two row-tiled streams: A on PE rows 0-49 (l-half 0),
                # B on rows 64-113 (l-half 1).
                tiles = [
                    pw.tile([128, 1024], FP32, name="pwm"),
                    pw.tile([128, 1024], FP32, name="pwm"),
                ]
                for jj in range(2):
                    nc.tensor.matmul(
                        tiles[0][:, ts(jj, 512)],
                        kq2[0:KQ, ts(m, 128)],
                        kqt[0:KQ, ts(jj, 512)],
                        start=True,
                        stop=True,
                    )
                    nc.tensor.matmul(
                        tiles[1][:, ts(jj, 512)],
                        kqt[64 : 64 + KQ, ts(m, 128)],
                        kq2[64 : 64 + KQ, ts(2 + jj, 512)],
                        start=True,
                        stop=True,
                    )
                return tiles

            pw2 = emit_mm2(0)
            if b == 0:
                for t in tasks0[6:]:
                    t()
            ti = 0

            def emit_mm3(m, vp, eh0_ap, eh1_ap):
                # y^T accum: quadrant selects partition half; order
                # 0,2,1,3 pairs different col-groups for concurrency
                for j in (0, 2, 1, 3):
                    eh = eh0_ap if j < 2 else eh1_ap
                    nc.tensor.matmul(
                        pyt[64 * (j // 2) : 64 * (j // 2) + C, ts(j % 2, 512)],
                        vp[:],
                        eh[:, ts(j % 2, 512)],
                        start=(m == 0),
                        stop=(m == NM - 1),
                    )

            for m in range(NM):
                sa = spool.tile([128, 1], FP32)
                sh = spool.tile([128, 1], FP32)
                eh0 = epool.tile([128, 1024], BF16)
                # h0 on ACT (native exp, free row-sum via accumulator)
                nc.scalar.activation(
                    eh0[:], pw2[0][:], AF.Exp, scale=scale, accum_out=sa[:]
                )
                if (b * 16 + m) not in DVE_M:
                    eh1 = epool.tile([128, 1024], BF16)
                    nc.scalar.activation(
                        eh1[:], pw2[1][:], AF.Exp, scale=scale, accum_out=sh[:]
                    )
                    eh1_ap = eh1[:]
                else:
                    # h1 on DVE: Schraudolph exp in bf16 bit space; the
                    # int16 bits ARE the bf16 et values, so mm3 reads the
                    # bitcast tile and only the row-sum needs a second op
                    xq = xqpool.tile([128, 1024], I16)
                    nc.vector.tensor_scalar(
                        xq[:], pw2[1][:], SA16 * scale, SB16, OP.mult, OP.add,
                    )
                    eh1_ap = xq[:].bitcast(BF16)
                    nc.vector.tensor_reduce(sh[:], eh1_ap, AX.X, OP.add)
                if m + 1 < NM:
                    pw2 = emit_mm2(m + 1)
                dsum = spool.tile([128, 1], FP32)
                nc.vector.tensor_add(dsum[:], sa[:], sh[:])
                dinv = spool.tile([128, 1], FP32)
                nc.vector.reciprocal(dinv[:], dsum[:])
                vp = vppool.tile([128, C], BF16)
                nc.vector.tensor_scalar_mul(vp[:], vsb[:, ts(m, C)], dinv[:])
                emit_mm3(m, vp[:], eh0[:], eh1_ap)
                if ti < len(tasks) and m >= 2:
                    tasks[ti]()
                    ti += 1

            while ti < len(tasks):
                tasks[ti]()
                ti += 1

            yt = ytpool.tile([128, 1024], FP32)
            nc.scalar.copy(yt[:], pyt[:])
            nc.sync.dma_start(y_d[b][0], yt[0:C, :])
            nc.sync.dma_start(y_d[b][1], yt[64 : 64 + C, :])

            if b + 1 < BPC:
                kqt, kq2, vsb = kqt_n, kq2_n, vsb_n

    nc.finalize()
    return nc


def kernel(x, Wk, bk, Wq, bq, Wv, bv, sample_len):
    global LAST_EXEC_NS
    from concourse.bass_utils import run_bass_kernel_spmd

    scale = float(1.0 / np.sqrt(np.float64(sample_len)))
    if scale not in _cache:
        _cache[scale] = _build(scale)
    nc = _cache[scale]

    import ml_dtypes

    bf16 = ml_dtypes.bfloat16
    x = np.asarray(x, dtype=np.float32)
    ones = np.ones((B, 1, L), dtype=np.float32)
    x = np.ascontiguousarray(np.concatenate([x, ones], axis=1)).astype(bf16)
    wkq = np.zeros((C + 1, 128), dtype=np.float32)
    wkq[:, 0:KQ] = np.concatenate([Wk, bk[None, :]], axis=0)
    wkq[:, 64 : 64 + KQ] = np.concatenate([Wq, bq[None, :]], axis=0)
    wkq = wkq.astype(bf16)
    wv = np.concatenate([Wv, bv[None, :]], axis=0).astype(bf16)

    in_maps = [
        {"x": x[i * BPC : (i + 1) * BPC], "wkq": wkq, "wv": wv}
        for i in range(NCORES)
    ]
    res = run_bass_kernel_spmd(nc, in_maps, list(range(NCORES)), trace=TRACE)
    LAST_EXEC_NS = res.exec_time_ns
    yp = np.concatenate([res.results[i]["y"] for i in range(NCORES)], axis=0)
    # yp: [B, 2, 64, 1024] -> y: [B, 2048, 64]
    y = yp.transpose(0, 1, 3, 2).reshape(B, L, C)
    return np.ascontiguousarray(y)


# revision 27
# speedup vs baseline: 1.2995x; 1.0149x over previous
import numpy as np

# nn_MyTemporalAttention: x [32, 64, 2048] -> y [32, 2048, 64]
B, C, L = 32, 64, 2048
KQ = 50
NCORES = 8
BPC = B // NCORES          # batches per core
NM = L // 128              # m-chunks of 128

TRACE = False
LAST_EXEC_NS = None
_cache = {}


def _build(scale: float):
    import concourse.bass as bass
    import concourse.tile as tile
    from concourse import bacc, mybir
    from contextlib import ExitStack

    FP32 = mybir.dt.float32
    BF16 = mybir.dt.bfloat16
    I32 = mybir.dt.int32
    AF = mybir.ActivationFunctionType
    OP = mybir.AluOpType
    ts = bass.ts

    # Schraudolph exp: bits = int32(SA*x + SB); valid since elu input <= 0
    SA = float(2**23 / np.log(2))
    SB = float(127 * 2**23 - 486408)

    nc = bacc.Bacc(
        "TRN2",
        target_bir_lowering=False,
        debug=False,
        enable_asserts=False,
        num_devices=NCORES,
    )
    # x host-augmented with ones row, bf16: [BPC, 65, L]
    x_d = nc.dram_tensor("x", [BPC, C + 1, L], BF16, kind="ExternalInput").ap()
    # wkq: [65, 128], cols 0-49 = [Wk;bk], cols 64-113 = [Wq;bq], rest zero
    wkq_d = nc.dram_tensor("wkq", [C + 1, 128], BF16, kind="ExternalInput").ap()
    # wv = [Wv; bv] -> [65, 64]
    wv_d = nc.dram_tensor("wv", [C + 1, C], BF16, kind="ExternalInput").ap()
    # y packed: [BPC, 2, 64, 1024]; [b, h, c, j] = y[b, h*1024+j, c]
    y_d = nc.dram_tensor("y", [BPC, 2, C, 1024], FP32, kind="ExternalOutput").ap()

    with tile.TileContext(nc) as tc, ExitStack() as ctx:
        const = ctx.enter_context(tc.tile_pool(name="const", bufs=1))
        xpool = ctx.enter_context(tc.tile_pool(name="xp", bufs=BPC))
        kqpool = ctx.enter_context(tc.tile_pool(name="kqt", bufs=2))
        kq2pool = ctx.enter_context(tc.tile_pool(name="kq2", bufs=2))
        xmpool = ctx.enter_context(tc.tile_pool(name="xm", bufs=2))
        xepool = ctx.enter_context(tc.tile_pool(name="xe", bufs=2))
        vpool = ctx.enter_context(tc.tile_pool(name="v", bufs=2))
        epool = ctx.enter_context(tc.tile_pool(name="e", bufs=6))
        ytpool = ctx.enter_context(tc.tile_pool(name="yt", bufs=2))
        spool = ctx.enter_context(tc.tile_pool(name="stats", bufs=12))
        vppool = ctx.enter_context(tc.tile_pool(name="vp", bufs=6))
        scrpool = ctx.enter_context(tc.tile_pool(name="scr", bufs=2))
        # PSUM: pw 3 x [128,1024] f32 (6 banks) + py 1 x [128,1024] f32 (2)
        pw = ctx.enter_context(tc.tile_pool(name="pw", bufs=3, space="PSUM"))
        py = ctx.enter_context(tc.tile_pool(name="py", bufs=1, space="PSUM"))

        # trigger the exp table-set load early so it overlaps the prologue
        warm = const.tile([1, 2], FP32)
        nc.vector.memset(warm[0:1, 0:1], 0.0)
        nc.scalar.activation(warm[0:1, 1:2], warm[0:1, 0:1], AF.Exp)

        wkq = const.tile([C + 1, 128], BF16)
        nc.sync.dma_start(wkq[:], wkq_d[:])
        # x0 right after wkq: it gates the first kq matmul
        xps = []
        for b in range(BPC):
            xp = xpool.tile([C + 1, L], BF16)
            xps.append(xp)
        nc.sync.dma_start(xps[0][:], x_d[0])
        wv = const.tile([C + 1, C], BF16)
        nc.sync.dma_start(wv[:], wv_d[:])
        for b in range(1, BPC):
            nc.sync.dma_start(xps[b][:], x_d[b])

        def prep_tasks(b):
            """Emission closures producing kqt/kq2/vsb for batch b."""
            xp = xps[b]
            # kqt: k rows at partitions 0-49, q rows at partitions 64-113
            # kq2 (mirror): q rows at partitions 0-49, k rows at 64-113
            kqt = kqpool.tile([128, L], BF16)
            kq2 = kq2pool.tile([128, L], BF16)
            vsb = vpool.tile([128, NM * C], BF16)
            pkqs = [None, None]

            def kq_mm(h):
                pkq = pw.tile([128, 1024], FP32, name="pwm")
                pkqs[h] = pkq
                for j in range(2):
                    nc.tensor.matmul(
                        pkq[:, ts(j, 512)], wkq[:], xp[:, ts(2 * h + j, 512)],
                        start=True, stop=True,
                    )

            def elu(h):
                pkq = pkqs[h]
                xm = xmpool.tile([128, 1024], BF16)
                nc.vector.tensor_scalar_min(xm[:], pkq[:], 0.0)
                xe = xepool.tile([128, 1024], I32)
                nc.vector.tensor_scalar(xe[:], xm[:], SA, SB, OP.mult, OP.add)
                nc.vector.scalar_tensor_tensor(
                    kqt[:, ts(h, 1024)], xe[:].bitcast(FP32), -1.0, pkq[:],
                    OP.add, OP.max,
                )

            def qdup():
                nc.sync.dma_start(kq2[0:KQ, :], kqt[64 : 64 + KQ, :])
                nc.sync.dma_start(kq2[64 : 64 + KQ, :], kqt[0:KQ, :])

            def qdup_half(h):
                sl = slice(1024 * h, 1024 * (h + 1))
                nc.sync.dma_start(kq2[0:KQ, sl], kqt[64 : 64 + KQ, sl])
                nc.sync.dma_start(kq2[64 : 64 + KQ, sl], kqt[0:KQ, sl])

            pvs = [None]

            def v_mm(part):
                if part == 0:
                    pvs[0] = pw.tile([128, 1024], FP32, name="pwm")
                pv = pvs[0]
                for jj in range(8 * part, 8 * part + 8):
                    nc.tensor.matmul(
                        pv[:, ts(jj, C)], xp[:, ts(jj, 128)], wv[:],
                        start=True, stop=True,
                    )

            def v_tanh():
                nc.scalar.activation(vsb[:], pvs[0][:], AF.Tanh)

            # first 10 tasks are the critical chain to the first exp of the
            # batch; the rest can lag
            if b == 0:
                # batch 0 runs fully upfront: split qdup per l-half so the
                # first mm2/exp only waits on elu(h0), not both halves
                tasks = [
                    lambda: kq_mm(0),
                    lambda: elu(0),
                    lambda: qdup_half(0),
                    lambda: kq_mm(1),
                    lambda: elu(1),
                    lambda: qdup_half(1),
                    lambda: v_mm(0),
                    lambda: v_mm(1),
                    v_tanh,
                ]
            else:
                tasks = []
                for h in range(2):
                    tasks.append(lambda h=h: kq_mm(h))
                    tasks.append(lambda h=h: elu(h))
                tasks.append(qdup)
                tasks.append(lambda: v_mm(0))
                tasks.append(lambda: v_mm(1))
                tasks.append(v_tanh)
            return kqt, kq2, vsb, tasks

        # PE warmup: ~4.5us of back-to-back dummy matmuls during the DMA
        # prologue (PE is otherwise idle) to engage the HAM 8/8 clock
        wz = const.tile([128, 512], BF16)
        nc.vector.memset(wz[:], 0.0)
        # warm the DVE cache-reduce uop path (first use gives a bad sum)
        wd = const.tile([128, 1], FP32)
        nc.vector.tensor_scalar(
            wz[:, 0:64], wz[:, 0:64], 1.0, 0.0, OP.mult, OP.add,
            accum_out=wd[:],
        )
        pwarm = pw.tile([128, 1024], FP32, name="pwm")
        for r in range(10):
            nc.tensor.matmul(
                pwarm[:, 0:512], wz[:, 0:128], wz[:],
                start=True, stop=True,
            )

        kqt, kq2, vsb, tasks0 = prep_tasks(0)
        for t in tasks0[:6]:
            t()

        # dummy matmuls to keep the PE busy while the DVE computes batch-0
        # elu: the ~6.7us idle gap here otherwise re-throttles the HAM
        # clock gate to K=4/8 for the rest of the kernel
        dumt = pw.tile([128, 1024], FP32, name="pwm")
        for r in range(44):
            nc.tensor.matmul(
                dumt[:, ts(r % 2, 512)], wz[:, 0:128], wz[:],
                start=True, stop=True,
            )

        for b in range(BPC):
            if b + 1 < BPC:
                kqt_n, kq2_n, vsb_n, tasks = prep_tasks(b + 1)
            else:
                tasks = []

            pyt = py.tile([128, 1024], FP32, name="pyt")

            def emit_mm2(m):
                # Two concurrent row-tiled streams: A on PE rows 0-49
                # (l-half 0), B on rows 64-113 (l-half 1).
                tiles = [
                    pw.tile([128, 1024], FP32, name="pwm"),
                    pw.tile([128, 1024], FP32, name="pwm"),
                ]
                for jj in range(2):
                    nc.tensor.matmul(
                        tiles[0][:, ts(jj, 512)],
                        kq2[0:KQ, ts(m, 128)],
                        kqt[0:KQ, ts(jj, 512)],
                        start=True,
                        stop=True,
                    )
                    nc.tensor.matmul(
                        tiles[1][:, ts(jj, 512)],
                        kqt[64 : 64 + KQ, ts(m, 128)],
                        kq2[64 : 64 + KQ, ts(2 + jj, 512)],
                        start=True,
                        stop=True,
                    )
                return tiles

            pw2 = emit_mm2(0)
            if b == 0:
                # batch-0 v/tanh after mm2(0): keeps the first exps off the
                # back of 16 v-matmuls on the PE queue
                for t in tasks0[6:]:
                    t()
            ti = 0

            def emit_mm3(m, vp, et):
                # y^T accum: quadrant (j//2) selects partition half; order
                # 0,2,1,3 pairs different col-groups for concurrency
                for j in (0, 2, 1, 3):
                    nc.tensor.matmul(
                        pyt[64 * (j // 2) : 64 * (j // 2) + C, ts(j % 2, 512)],
                        vp[:],
                        et[:, ts(j, 512)],
                        start=(m == 0),
                        stop=(m == NM - 1),
                    )

            for m in range(NM):
                et = epool.tile([128, L], BF16)
                d2 = spool.tile([128, 2], FP32)
                for h in range(2):
                    nc.scalar.activation(
                        et[:, ts(h, 1024)], pw2[h][:], AF.Exp, scale=scale,
                        accum_out=d2[:, h : h + 1],
                    )
                if m + 1 < NM:
                    pw2 = emit_mm2(m + 1)
                dsum = spool.tile([128, 1], FP32)
                nc.vector.tensor_add(dsum[:], d2[:, 0:1], d2[:, 1:2])
                dinv = spool.tile([128, 1], FP32)
                nc.vector.reciprocal(dinv[:], dsum[:])
                vp = vppool.tile([128, C], BF16)
                nc.vector.tensor_scalar_mul(vp[:], vsb[:, ts(m, C)], dinv[:])
                emit_mm3(m, vp, et)
                if ti < len(tasks) and m >= 2:
                    tasks[ti]()
                    ti += 1

            while ti < len(tasks):
                tasks[ti]()
                ti += 1

            yt = ytpool.tile([128, 1024], FP32)
            nc.vector.tensor_copy(yt[:], pyt[:])
            nc.sync.dma_start(y_d[b][0], yt[0:C, :])
            nc.sync.dma_start(y_d[b][1], yt[64 : 64 + C, :])

            if b + 1 < BPC:
                kqt, kq2, vsb = kqt_n, kq2_n, vsb_n

    nc.finalize()
    return nc


def kernel(x, Wk, bk, Wq, bq, Wv, bv, sample_len):
    global LAST_EXEC_NS
    from concourse.bass_utils import run_bass_kernel_spmd

    scale = float(1.0 / np.sqrt(np.float64(sample_len)))
    if scale not in _cache:
        _cache[scale] = _build(scale)
    nc = _cache[scale]

    import ml_dtypes

    bf16 = ml_dtypes.bfloat16
    x = np.asarray(x, dtype=np.float32)
    ones = np.ones((B, 1, L), dtype=np.float32)
    x = np.ascontiguousarray(np.concatenate([x, ones], axis=1)).astype(bf16)
    wkq = np.zeros((C + 1, 128), dtype=np.float32)
    wkq[:, 0:KQ] = np.concatenate([Wk, bk[None, :]], axis=0)
    wkq[:, 64 : 64 + KQ] = np.concatenate([Wq, bq[None, :]], axis=0)
    wkq = wkq.astype(bf16)
    wv = np.concatenate([Wv, bv[None, :]], axis=0).astype(bf16)

    in_maps = [
        {"x": x[i * BPC : (i + 1) * BPC], "wkq": wkq, "wv": wv}
        for i in range(NCORES)
    ]
    res = run_bass_kernel_spmd(nc, in_maps, list(range(NCORES)), trace=TRACE)
    LAST_EXEC_NS = res.exec_time_ns
    yp = np.concatenate([res.results[i]["y"] for i in range(NCORES)], axis=0)
    # yp: [B, 2, 64, 1024] -> y: [B, 2048, 64]
    y = yp.transpose(0, 1, 3, 2).reshape(B, L, C)
    return np.ascontiguousarray(y)


# revision 28
# speedup vs baseline: 1.3165x; 1.0131x over previous
import numpy as np

# nn_MyTemporalAttention: x [32, 64, 2048] -> y [32, 2048, 64]
B, C, L = 32, 64, 2048
KQ = 50
NCORES = 8
BPC = B // NCORES          # batches per core
NM = L // 128              # m-chunks of 128

TRACE = False
LAST_EXEC_NS = None
_cache = {}


def _build(scale: float):
    import concourse.bass as bass
    import concourse.tile as tile
    from concourse import bacc, mybir
    from contextlib import ExitStack

    FP32 = mybir.dt.float32
    BF16 = mybir.dt.bfloat16
    I32 = mybir.dt.int32
    AF = mybir.ActivationFunctionType
    OP = mybir.AluOpType
    ts = bass.ts

    # Schraudolph exp: bits = int32(SA*x + SB); valid since elu input <= 0
    SA = float(2**23 / np.log(2))
    SB = float(127 * 2**23 - 486408)

    nc = bacc.Bacc(
        "TRN2",
        target_bir_lowering=False,
        debug=False,
        enable_asserts=False,
        num_devices=NCORES,
    )
    # x host-augmented with ones row, bf16: [BPC, 65, L]
    x_d = nc.dram_tensor("x", [BPC, C + 1, L], BF16, kind="ExternalInput").ap()
    # wkq: [65, 128], cols 0-49 = [Wk;bk], cols 64-113 = [Wq;bq], rest zero
    wkq_d = nc.dram_tensor("wkq", [C + 1, 128], BF16, kind="ExternalInput").ap()
    # wv = [Wv; bv] -> [65, 64]
    wv_d = nc.dram_tensor("wv", [C + 1, C], BF16, kind="ExternalInput").ap()
    # y packed: [BPC, 2, 64, 1024]; [b, h, c, j] = y[b, h*1024+j, c]
    y_d = nc.dram_tensor("y", [BPC, 2, C, 1024], FP32, kind="ExternalOutput").ap()

    with tile.TileContext(nc) as tc, ExitStack() as ctx:
        const = ctx.enter_context(tc.tile_pool(name="const", bufs=1))
        xpool = ctx.enter_context(tc.tile_pool(name="xp", bufs=BPC))
        kqpool = ctx.enter_context(tc.tile_pool(name="kqt", bufs=2))
        kq2pool = ctx.enter_context(tc.tile_pool(name="kq2", bufs=2))
        xmpool = ctx.enter_context(tc.tile_pool(name="xm", bufs=2))
        xepool = ctx.enter_context(tc.tile_pool(name="xe", bufs=2))
        vpool = ctx.enter_context(tc.tile_pool(name="v", bufs=2))
        epool = ctx.enter_context(tc.tile_pool(name="e", bufs=6))
        ytpool = ctx.enter_context(tc.tile_pool(name="yt", bufs=2))
        spool = ctx.enter_context(tc.tile_pool(name="stats", bufs=12))
        vppool = ctx.enter_context(tc.tile_pool(name="vp", bufs=6))
        scrpool = ctx.enter_context(tc.tile_pool(name="scr", bufs=2))
        # PSUM: pw 3 x [128,1024] f32 (6 banks) + py 1 x [128,1024] f32 (2)
        pw = ctx.enter_context(tc.tile_pool(name="pw", bufs=3, space="PSUM"))
        py = ctx.enter_context(tc.tile_pool(name="py", bufs=1, space="PSUM"))

        # trigger the exp table-set load early so it overlaps the prologue
        warm = const.tile([1, 2], FP32)
        nc.vector.memset(warm[0:1, 0:1], 0.0)
        nc.scalar.activation(warm[0:1, 1:2], warm[0:1, 0:1], AF.Exp)

        wkq = const.tile([C + 1, 128], BF16)
        nc.sync.dma_start(wkq[:], wkq_d[:])
        # x0 right after wkq: it gates the first kq matmul
        xps = []
        for b in range(BPC):
            xp = xpool.tile([C + 1, L], BF16)
            xps.append(xp)
        nc.sync.dma_start(xps[0][:], x_d[0])
        wv = const.tile([C + 1, C], BF16)
        nc.sync.dma_start(wv[:], wv_d[:])
        for b in range(1, BPC):
            nc.sync.dma_start(xps[b][:], x_d[b])

        def prep_tasks(b):
            """Emission closures producing kqt/kq2/vsb for batch b."""
            xp = xps[b]
            # kqt: k rows at partitions 0-49, q rows at partitions 64-113
            # kq2 (mirror): q rows at partitions 0-49, k rows at 64-113
            kqt = kqpool.tile([128, L], BF16)
            kq2 = kq2pool.tile([128, L], BF16)
            vsb = vpool.tile([128, NM * C], BF16)
            pkqs = [None, None]

            def kq_mm(h):
                pkq = pw.tile([128, 1024], FP32, name="pwm")
                pkqs[h] = pkq
                for j in range(2):
                    nc.tensor.matmul(
                        pkq[:, ts(j, 512)], wkq[:], xp[:, ts(2 * h + j, 512)],
                        start=True, stop=True,
                    )

            def elu(h):
                pkq = pkqs[h]
                xm = xmpool.tile([128, 1024], BF16)
                nc.vector.tensor_scalar_min(xm[:], pkq[:], 0.0)
                xe = xepool.tile([128, 1024], I32)
                nc.vector.tensor_scalar(xe[:], xm[:], SA, SB, OP.mult, OP.add)
                nc.vector.scalar_tensor_tensor(
                    kqt[:, ts(h, 1024)], xe[:].bitcast(FP32), -1.0, pkq[:],
                    OP.add, OP.max,
                )

            def qdup():
                nc.sync.dma_start(kq2[0:KQ, :], kqt[64 : 64 + KQ, :])
                nc.sync.dma_start(kq2[64 : 64 + KQ, :], kqt[0:KQ, :])

            def qdup_half(h):
                sl = slice(1024 * h, 1024 * (h + 1))
                nc.sync.dma_start(kq2[0:KQ, sl], kqt[64 : 64 + KQ, sl])
                nc.sync.dma_start(kq2[64 : 64 + KQ, sl], kqt[0:KQ, sl])

            pvs = [None]

            def v_mm(part):
                if part == 0:
                    pvs[0] = pw.tile([128, 1024], FP32, name="pwm")
                pv = pvs[0]
                for jj in range(8 * part, 8 * part + 8):
                    nc.tensor.matmul(
                        pv[:, ts(jj, C)], xp[:, ts(jj, 128)], wv[:],
                        start=True, stop=True,
                    )

            def v_tanh():
                nc.scalar.activation(vsb[:], pvs[0][:], AF.Tanh)

            # first 10 tasks are the critical chain to the first exp of the
            # batch; the rest can lag
            if b == 0:
                # batch 0 runs fully upfront: split qdup per l-half so the
                # first mm2/exp only waits on elu(h0), not both halves
                tasks = [
                    lambda: kq_mm(0),
                    lambda: elu(0),
                    lambda: qdup_half(0),
                    lambda: kq_mm(1),
                    lambda: elu(1),
                    lambda: qdup_half(1),
                    lambda: v_mm(0),
                    lambda: v_mm(1),
                    v_tanh,
                ]
            else:
                tasks = []
                for h in range(2):
                    tasks.append(lambda h=h: kq_mm(h))
                    tasks.append(lambda h=h: elu(h))
                tasks.append(qdup)
                tasks.append(lambda: v_mm(0))
                tasks.append(lambda: v_mm(1))
                tasks.append(v_tanh)
            return kqt, kq2, vsb, tasks

        # PE warmup: ~4.5us of back-to-back dummy matmuls during the DMA
        # prologue (PE is otherwise idle) to engage the HAM 8/8 clock
        wz = const.tile([128, 512], BF16)
        nc.vector.memset(wz[:], 0.0)
        # warm the DVE cache-reduce uop path (first use gives a bad sum)
        wd = const.tile([128, 1], FP32)
        nc.vector.tensor_scalar(
            wz[:, 0:64], wz[:, 0:64], 1.0, 0.0, OP.mult, OP.add,
            accum_out=wd[:],
        )
        pwarm = pw.tile([128, 1024], FP32, name="pwm")
        for r in range(10):
            nc.tensor.matmul(
                pwarm[:, 0:512], wz[:, 0:128], wz[:],
                start=True, stop=True,
            )

        kqt, kq2, vsb, tasks0 = prep_tasks(0)
        for t in tasks0[:6]:
            t()

        for b in range(BPC):
            if b + 1 < BPC:
                kqt_n, kq2_n, vsb_n, tasks = prep_tasks(b + 1)
            else:
                tasks = []

            pyt = py.tile([128, 1024], FP32, name="pyt")

            def emit_mm2(m):
                # Two concurrent row-tiled streams: A on PE rows 0-49
                # (l-half 0), B on rows 64-113 (l-half 1).
                tiles = [
                    pw.tile([128, 1024], FP32, name="pwm"),
                    pw.tile([128, 1024], FP32, name="pwm"),
                ]
                for jj in range(2):
                    nc.tensor.matmul(
                        tiles[0][:, ts(jj, 512)],
                        kq2[0:KQ, ts(m, 128)],
                        kqt[0:KQ, ts(jj, 512)],
                        start=True,
                        stop=True,
                    )
                    nc.tensor.matmul(
                        tiles[1][:, ts(jj, 512)],
                        kqt[64 : 64 + KQ, ts(m, 128)],
                        kq2[64 : 64 + KQ, ts(2 + jj, 512)],
                        start=True,
                        stop=True,
                    )
                return tiles

            pw2 = emit_mm2(0)
            if b == 0:
                # batch-0 v/tanh after mm2(0): keeps the first exps off the
                # back of 16 v-matmuls on the PE queue
                for t in tasks0[6:]:
                    t()
            ti = 0

            def emit_mm3(m, vp, et):
                # y^T accum: quadrant (j//2) selects partition half; order
                # 0,2,1,3 pairs different col-groups for concurrency
                for j in (0, 2, 1, 3):
                    nc.tensor.matmul(
                        pyt[64 * (j // 2) : 64 * (j // 2) + C, ts(j % 2, 512)],
                        vp[:],
                        et[:, ts(j, 512)],
                        start=(m == 0),
                        stop=(m == NM - 1),
                    )

            for m in range(NM):
                et = epool.tile([128, L], BF16)
                d2 = spool.tile([128, 2], FP32)
                for h in range(2):
                    nc.scalar.activation(
                        et[:, ts(h, 1024)], pw2[h][:], AF.Exp, scale=scale,
                        accum_out=d2[:, h : h + 1],
                    )
                if m + 1 < NM:
                    pw2 = emit_mm2(m + 1)
                dsum = spool.tile([128, 1], FP32)
                nc.vector.tensor_add(dsum[:], d2[:, 0:1], d2[:, 1:2])
                dinv = spool.tile([128, 1], FP32)
                nc.vector.reciprocal(dinv[:], dsum[:])
                vp = vppool.tile([128, C], BF16)
                nc.vector.tensor_scalar_mul(vp[:], vsb[:, ts(m, C)], dinv[:])
                emit_mm3(m, vp, et)
                if ti < len(tasks) and m >= 2:
                    tasks[ti]()
                    ti += 1

            while ti < len(tasks):
                tasks[ti]()
                ti += 1

            yt = ytpool.tile([128, 1024], FP32)
            nc.vector.tensor_copy(yt[:], pyt[:])
            nc.sync.dma_start(y_d[b][0], yt[0:C, :])
            nc.sync.dma_start(y_d[b][1], yt[64 : 64 + C, :])

            if b + 1 < BPC:
                kqt, kq2, vsb = kqt_n, kq2_n, vsb_n

    nc.finalize()
    return nc


def kernel(x, Wk, bk, Wq, bq, Wv, bv, sample_len):
    global LAST_EXEC_NS
    from concourse.bass_utils import run_bass_kernel_spmd

    scale = float(1.0 / np.sqrt(np.float64(sample_len)))
    if scale not in _cache:
        _cache[scale] = _build(scale)
    nc = _cache[scale]

    import ml_dtypes

    bf16 = ml_dtypes.bfloat16
    x = np.asarray(x, dtype=np.float32)
    ones = np.ones((B, 1, L), dtype=np.float32)
    x = np.ascontiguousarray(np.concatenate([x, ones], axis=1)).astype(bf16)
    wkq = np.zeros((C + 1, 128), dtype=np.float32)
    wkq[:, 0:KQ] = np.concatenate([Wk, bk[None, :]], axis=0)
    wkq[:, 64 : 64 + KQ] = np.concatenate([Wq, bq[None, :]], axis=0)
    wkq = wkq.astype(bf16)
    wv = np.concatenate([Wv, bv[None, :]], axis=0).astype(bf16)

    in_maps = [
        {"x": x[i * BPC : (i + 1) * BPC], "wkq": wkq, "wv": wv}
        for i in range(NCORES)
    ]
    res = run_bass_kernel_spmd(nc, in_maps, list(range(NCORES)), trace=TRACE)
    LAST_EXEC_NS = res.exec_time_ns
    yp = np.concatenate([res.results[i]["y"] for i in range(NCORES)], axis=0)
    # yp: [B, 2, 64, 1024] -> y: [B, 2048, 64]
    y = yp.transpose(0, 1, 3, 2).reshape(B, L, C)
    return np.ascontiguousarray(y)


# revision 30
# speedup vs baseline: 1.3168x; 1.0003x over previous
import numpy as np

# nn_MyTemporalAttention: x [32, 64, 2048] -> y [32, 2048, 64]
B, C, L = 32, 64, 2048
KQ = 50
NCORES = 8
BPC = B // NCORES          # batches per core
NM = L // 128              # m-chunks of 128

TRACE = False
LAST_EXEC_NS = None
_cache = {}


def _build(scale: float):
    import concourse.bass as bass
    import concourse.tile as tile
    from concourse import bacc, mybir
    from contextlib import ExitStack

    FP32 = mybir.dt.float32
    BF16 = mybir.dt.bfloat16
    I32 = mybir.dt.int32
    AF = mybir.ActivationFunctionType
    OP = mybir.AluOpType
    ts = bass.ts

    # Schraudolph exp: bits = int32(SA*x + SB); valid since elu input <= 0
    SA = float(2**23 / np.log(2))
    SB = float(127 * 2**23 - 486408)

    nc = bacc.Bacc(
        "TRN2",
        target_bir_lowering=False,
        debug=False,
        enable_asserts=False,
        num_devices=NCORES,
    )
    # x host-augmented with ones row, bf16: [BPC, 65, L]
    x_d = nc.dram_tensor("x", [BPC, C + 1, L], BF16, kind="ExternalInput").ap()
    # wkq: [65, 128], cols 0-49 = [Wk;bk], cols 64-113 = [Wq;bq], rest zero
    wkq_d = nc.dram_tensor("wkq", [C + 1, 128], BF16, kind="ExternalInput").ap()
    # wv = [Wv; bv] -> [65, 64]
    wv_d = nc.dram_tensor("wv", [C + 1, C], BF16, kind="ExternalInput").ap()
    # y packed: [BPC, 2, 64, 1024]; [b, h, c, j] = y[b, h*1024+j, c]
    y_d = nc.dram_tensor("y", [BPC, 2, C, 1024], FP32, kind="ExternalOutput").ap()

    with tile.TileContext(nc) as tc, ExitStack() as ctx:
        const = ctx.enter_context(tc.tile_pool(name="const", bufs=1))
        xpool = ctx.enter_context(tc.tile_pool(name="xp", bufs=BPC))
        kqpool = ctx.enter_context(tc.tile_pool(name="kqt", bufs=2))
        kq2pool = ctx.enter_context(tc.tile_pool(name="kq2", bufs=2))
        xmpool = ctx.enter_context(tc.tile_pool(name="xm", bufs=2))
        xepool = ctx.enter_context(tc.tile_pool(name="xe", bufs=2))
        vpool = ctx.enter_context(tc.tile_pool(name="v", bufs=2))
        epool = ctx.enter_context(tc.tile_pool(name="e", bufs=6))
        ytpool = ctx.enter_context(tc.tile_pool(name="yt", bufs=2))
        spool = ctx.enter_context(tc.tile_pool(name="stats", bufs=12))
        vppool = ctx.enter_context(tc.tile_pool(name="vp", bufs=6))
        scrpool = ctx.enter_context(tc.tile_pool(name="scr", bufs=2))
        # PSUM: pw 3 x [128,1024] f32 (6 banks) + py 1 x [128,1024] f32 (2)
        pw = ctx.enter_context(tc.tile_pool(name="pw", bufs=3, space="PSUM"))
        py = ctx.enter_context(tc.tile_pool(name="py", bufs=1, space="PSUM"))

        # trigger the exp table-set load early so it overlaps the prologue
        warm = const.tile([1, 2], FP32)
        nc.vector.memset(warm[0:1, 0:1], 0.0)
        nc.scalar.activation(warm[0:1, 1:2], warm[0:1, 0:1], AF.Exp)

        wkq = const.tile([C + 1, 128], BF16)
        nc.sync.dma_start(wkq[:], wkq_d[:])
        # x0 right after wkq: it gates the first kq matmul
        xps = []
        for b in range(BPC):
            xp = xpool.tile([C + 1, L], BF16)
            xps.append(xp)
        nc.sync.dma_start(xps[0][:], x_d[0])
        wv = const.tile([C + 1, C], BF16)
        nc.sync.dma_start(wv[:], wv_d[:])
        for b in range(1, BPC):
            nc.sync.dma_start(xps[b][:], x_d[b])

        def prep_tasks(b):
            """Emission closures producing kqt/kq2/vsb for batch b."""
            xp = xps[b]
            # kqt: k rows at partitions 0-49, q rows at partitions 64-113
            # kq2 (mirror): q rows at partitions 0-49, k rows at 64-113
            kqt = kqpool.tile([128, L], BF16)
            kq2 = kq2pool.tile([128, L], BF16)
            vsb = vpool.tile([128, NM * C], BF16)
            pkqs = [None, None]

            def kq_mm(h):
                pkq = pw.tile([128, 1024], FP32, name="pwm")
                pkqs[h] = pkq
                for j in range(2):
                    nc.tensor.matmul(
                        pkq[:, ts(j, 512)], wkq[:], xp[:, ts(2 * h + j, 512)],
                        start=True, stop=True,
                    )

            def elu(h):
                pkq = pkqs[h]
                xm = xmpool.tile([128, 1024], BF16)
                nc.vector.tensor_scalar_min(xm[:], pkq[:], 0.0)
                xe = xepool.tile([128, 1024], I32)
                nc.vector.tensor_scalar(xe[:], xm[:], SA, SB, OP.mult, OP.add)
                nc.vector.scalar_tensor_tensor(
                    kqt[:, ts(h, 1024)], xe[:].bitcast(FP32), -1.0, pkq[:],
                    OP.add, OP.max,
                )

            def qdup():
                nc.sync.dma_start(kq2[0:KQ, :], kqt[64 : 64 + KQ, :])
                nc.sync.dma_start(kq2[64 : 64 + KQ, :], kqt[0:KQ, :])

            def qdup_half(h):
                sl = slice(1024 * h, 1024 * (h + 1))
                nc.sync.dma_start(kq2[0:KQ, sl], kqt[64 : 64 + KQ, sl])
                nc.sync.dma_start(kq2[64 : 64 + KQ, sl], kqt[0:KQ, sl])

            pvs = [None]

            def v_mm(part):
                if part == 0:
                    pvs[0] = pw.tile([128, 1024], FP32, name="pwm")
                pv = pvs[0]
                for jj in range(8 * part, 8 * part + 8):
                    nc.tensor.matmul(
                        pv[:, ts(jj, C)], xp[:, ts(jj, 128)], wv[:],
                        start=True, stop=True,
                    )

            def v_tanh():
                nc.scalar.activation(vsb[:], pvs[0][:], AF.Tanh)

            # first 10 tasks are the critical chain to the first exp of the
            # batch; the rest can lag
            if b == 0:
                # batch 0 runs fully upfront: split qdup per l-half so the
                # first mm2/exp only waits on elu(h0), not both halves
                tasks = [
                    lambda: kq_mm(0),
                    lambda: elu(0),
                    lambda: qdup_half(0),
                    lambda: kq_mm(1),
                    lambda: elu(1),
                    lambda: qdup_half(1),
                    lambda: v_mm(0),
                    lambda: v_mm(1),
                    v_tanh,
                ]
            else:
                tasks = []
                for h in range(2):
                    tasks.append(lambda h=h: kq_mm(h))
                    tasks.append(lambda h=h: elu(h))
                tasks.append(qdup)
                tasks.append(lambda: v_mm(0))
                tasks.append(lambda: v_mm(1))
                tasks.append(v_tanh)
            return kqt, kq2, vsb, tasks

        # PE warmup: ~4.5us of back-to-back dummy matmuls during the DMA
        # prologue (PE is otherwise idle) to engage the HAM 8/8 clock
        wz = const.tile([128, 512], BF16)
        nc.vector.memset(wz[:], 0.0)
        # warm the DVE cache-reduce uop path (first use gives a bad sum)
        wd = const.tile([128, 1], FP32)
        nc.vector.tensor_scalar(
            wz[:, 0:64], wz[:, 0:64], 1.0, 0.0, OP.mult, OP.add,
            accum_out=wd[:],
        )
        pwarm = pw.tile([128, 1024], FP32, name="pwm")
        for r in range(10):
            nc.tensor.matmul(
                pwarm[:, 0:512], wz[:, 0:128], wz[:],
                start=True, stop=True,
            )

        kqt, kq2, vsb, tasks0 = prep_tasks(0)
        for t in tasks0[:6]:
            t()

        for b in range(BPC):
            if b + 1 < BPC:
                kqt_n, kq2_n, vsb_n, tasks = prep_tasks(b + 1)
            else:
                tasks = []

            pyt = py.tile([128, 1024], FP32, name="pyt")

            def emit_mm2(m):
                # Two concurrent row-tiled streams: A on PE rows 0-49
                # (l-half 0), B on rows 64-113 (l-half 1).
                tiles = [
                    pw.tile([128, 1024], FP32, name="pwm"),
                    pw.tile([128, 1024], FP32, name="pwm"),
                ]
                # B before A in each pair: tiles[1]'s PSUM buffer is freed
                # by the previous chunk's h1 exp, which (with h1-first exp
                # order below) completes before h0's, so stream B can start
                # while A still waits on its buffer
                for jj in range(2):
                    nc.tensor.matmul(
                        tiles[1][:, ts(jj, 512)],
                        kqt[64 : 64 + KQ, ts(m, 128)],
                        kq2[64 : 64 + KQ, ts(2 + jj, 512)],
                        start=True,
                        stop=True,
                    )
                    nc.tensor.matmul(
                        tiles[0][:, ts(jj, 512)],
                        kq2[0:KQ, ts(m, 128)],
                        kqt[0:KQ, ts(jj, 512)],
                        start=True,
                        stop=True,
                    )
                return tiles

            pw2 = emit_mm2(0)
            if b == 0:
                # batch-0 v/tanh after mm2(0): keeps the first exps off the
                # back of 16 v-matmuls on the PE queue
                for t in tasks0[6:]:
                    t()
            ti = 0

            def emit_mm3(m, vp, et):
                # y^T accum: quadrant (j//2) selects partition half; order
                # 0,2,1,3 pairs different col-groups for concurrency
                for j in (0, 2, 1, 3):
                    nc.tensor.matmul(
                        pyt[64 * (j // 2) : 64 * (j // 2) + C, ts(j % 2, 512)],
                        vp[:],
                        et[:, ts(j, 512)],
                        start=(m == 0),
                        stop=(m == NM - 1),
                    )

            for m in range(NM):
                et = epool.tile([128, L], BF16)
                d2 = spool.tile([128, 2], FP32)
                # h1 first: frees stream B's PSUM buffer earlier (see
                # emit_mm2 ordering note)
                for h in (1, 0):
                    nc.scalar.activation(
                        et[:, ts(h, 1024)], pw2[h][:], AF.Exp, scale=scale,
                        accum_out=d2[:, h : h + 1],
                    )
                if m + 1 < NM:
                    pw2 = emit_mm2(m + 1)
                dsum = spool.tile([128, 1], FP32)
                nc.vector.tensor_add(dsum[:], d2[:, 0:1], d2[:, 1:2])
                dinv = spool.tile([128, 1], FP32)
                nc.vector.reciprocal(dinv[:], dsum[:])
                vp = vppool.tile([128, C], BF16)
                nc.vector.tensor_scalar_mul(vp[:], vsb[:, ts(m, C)], dinv[:])
                emit_mm3(m, vp, et)
                if ti < len(tasks) and m >= 2:
                    tasks[ti]()
                    ti += 1

            while ti < len(tasks):
                tasks[ti]()
                ti += 1

            yt = ytpool.tile([128, 1024], FP32)
            nc.vector.tensor_copy(yt[:], pyt[:])
            nc.sync.dma_start(y_d[b][0], yt[0:C, :])
            nc.sync.dma_start(y_d[b][1], yt[64 : 64 + C, :])

            if b + 1 < BPC:
                kqt, kq2, vsb = kqt_n, kq2_n, vsb_n

    nc.finalize()
    return nc


def kernel(x, Wk, bk, Wq, bq, Wv, bv, sample_len):
    global LAST_EXEC_NS
    from concourse.bass_utils import run_bass_kernel_spmd

    scale = float(1.0 / np.sqrt(np.float64(sample_len)))
    if scale not in _cache:
        _cache[scale] = _build(scale)
    nc = _cache[scale]

    import ml_dtypes

    bf16 = ml_dtypes.bfloat16
    x = np.asarray(x, dtype=np.float32)
    ones = np.ones((B, 1, L), dtype=np.float32)
    x = np.ascontiguousarray(np.concatenate([x, ones], axis=1)).astype(bf16)
    wkq = np.zeros((C + 1, 128), dtype=np.float32)
    wkq[:, 0:KQ] = np.concatenate([Wk, bk[None, :]], axis=0)
    wkq[:, 64 : 64 + KQ] = np.concatenate([Wq, bq[None, :]], axis=0)
    wkq = wkq.astype(bf16)
    wv = np.concatenate([Wv, bv[None, :]], axis=0).astype(bf16)

    in_maps = [
        {"x": x[i * BPC : (i + 1) * BPC], "wkq": wkq, "wv": wv}
        for i in range(NCORES)
    ]
    res = run_bass_kernel_spmd(nc, in_maps, list(range(NCORES)), trace=TRACE)
    LAST_EXEC_NS = res.exec_time_ns
    yp = np.concatenate([res.results[i]["y"] for i in range(NCORES)], axis=0)
    # yp: [B, 2, 64, 1024] -> y: [B, 2048, 64]
    y = yp.transpose(0, 1, 3, 2).reshape(B, L, C)
    return np.ascontiguousarray(y)


# revision 32
# speedup vs baseline: 1.3226x; 1.0043x over previous
import numpy as np

# nn_MyTemporalAttention: x [32, 64, 2048] -> y [32, 2048, 64]
B, C, L = 32, 64, 2048
KQ = 50
NCORES = 8
BPC = B // NCORES          # batches per core
NM = L // 128              # m-chunks of 128

TRACE = False
LAST_EXEC_NS = None
_cache = {}


def _build(scale: float):
    import concourse.bass as bass
    import concourse.tile as tile
    from concourse import bacc, mybir
    from contextlib import ExitStack

    FP32 = mybir.dt.float32
    BF16 = mybir.dt.bfloat16
    I32 = mybir.dt.int32
    AF = mybir.ActivationFunctionType
    OP = mybir.AluOpType
    AX = mybir.AxisListType
    ts = bass.ts

    # Schraudolph exp: bits = int32(SA*x + SB); valid since elu input <= 0
    SA = float(2**23 / np.log(2))
    SB = float(127 * 2**23 - 486408)

    nc = bacc.Bacc(
        "TRN2",
        target_bir_lowering=False,
        debug=False,
        enable_asserts=False,
        num_devices=NCORES,
    )
    # x host-augmented with ones row, bf16: [BPC, 65, L]
    x_d = nc.dram_tensor("x", [BPC, C + 1, L], BF16, kind="ExternalInput").ap()
    # wkq: [65, 128], cols 0-49 = [Wk;bk], cols 64-113 = [Wq;bq], rest zero
    wkq_d = nc.dram_tensor("wkq", [C + 1, 128], BF16, kind="ExternalInput").ap()
    # wv = [Wv; bv] -> [65, 64]
    wv_d = nc.dram_tensor("wv", [C + 1, C], BF16, kind="ExternalInput").ap()
    # y packed: [BPC, 2, 64, 1024]; [b, h, c, j] = y[b, h*1024+j, c]
    y_d = nc.dram_tensor("y", [BPC, 2, C, 1024], FP32, kind="ExternalOutput").ap()

    with tile.TileContext(nc) as tc, ExitStack() as ctx:
        const = ctx.enter_context(tc.tile_pool(name="const", bufs=1))
        xpool = ctx.enter_context(tc.tile_pool(name="xp", bufs=BPC))
        kqpool = ctx.enter_context(tc.tile_pool(name="kqt", bufs=2))
        kq2pool = ctx.enter_context(tc.tile_pool(name="kq2", bufs=2))
        xmpool = ctx.enter_context(tc.tile_pool(name="xm", bufs=2))
        xepool = ctx.enter_context(tc.tile_pool(name="xe", bufs=2))
        vpool = ctx.enter_context(tc.tile_pool(name="v", bufs=2))
        epool = ctx.enter_context(tc.tile_pool(name="e", bufs=6))
        ytpool = ctx.enter_context(tc.tile_pool(name="yt", bufs=2))
        spool = ctx.enter_context(tc.tile_pool(name="stats", bufs=12))
        vppool = ctx.enter_context(tc.tile_pool(name="vp", bufs=6))
        scrpool = ctx.enter_context(tc.tile_pool(name="scr", bufs=2))
        # PSUM: pw 3 x [128,1024] f32 (6 banks) + py 1 x [128,1024] f32 (2)
        pw = ctx.enter_context(tc.tile_pool(name="pw", bufs=3, space="PSUM"))
        py = ctx.enter_context(tc.tile_pool(name="py", bufs=1, space="PSUM"))

        # trigger the exp table-set load early so it overlaps the prologue
        warm = const.tile([1, 2], FP32)
        nc.vector.memset(warm[0:1, 0:1], 0.0)
        nc.scalar.activation(warm[0:1, 1:2], warm[0:1, 0:1], AF.Exp)

        wkq = const.tile([C + 1, 128], BF16)
        nc.sync.dma_start(wkq[:], wkq_d[:])
        # x0 right after wkq: it gates the first kq matmul
        xps = []
        for b in range(BPC):
            xp = xpool.tile([C + 1, L], BF16)
            xps.append(xp)
        nc.sync.dma_start(xps[0][:], x_d[0])
        wv = const.tile([C + 1, C], BF16)
        nc.sync.dma_start(wv[:], wv_d[:])
        for b in range(1, BPC):
            nc.sync.dma_start(xps[b][:], x_d[b])

        def prep_tasks(b):
            """Emission closures producing kqt/kq2/vsb for batch b."""
            xp = xps[b]
            # kqt: k rows at partitions 0-49, q rows at partitions 64-113
            # kq2 (mirror): q rows at partitions 0-49, k rows at 64-113
            kqt = kqpool.tile([128, L], BF16)
            kq2 = kq2pool.tile([128, L], BF16)
            vsb = vpool.tile([128, NM * C], BF16)
            pkqs = [None, None]

            def kq_mm(h):
                pkq = pw.tile([128, 1024], FP32, name="pwm")
                pkqs[h] = pkq
                for j in range(2):
                    nc.tensor.matmul(
                        pkq[:, ts(j, 512)], wkq[:], xp[:, ts(2 * h + j, 512)],
                        start=True, stop=True,
                    )

            def elu(h):
                pkq = pkqs[h]
                xm = xmpool.tile([128, 1024], BF16)
                nc.vector.tensor_scalar_min(xm[:], pkq[:], 0.0)
                xe = xepool.tile([128, 1024], I32)
                nc.vector.tensor_scalar(xe[:], xm[:], SA, SB, OP.mult, OP.add)
                nc.vector.scalar_tensor_tensor(
                    kqt[:, ts(h, 1024)], xe[:].bitcast(FP32), -1.0, pkq[:],
                    OP.add, OP.max,
                )

            def qdup():
                nc.sync.dma_start(kq2[0:KQ, :], kqt[64 : 64 + KQ, :])
                nc.sync.dma_start(kq2[64 : 64 + KQ, :], kqt[0:KQ, :])

            def qdup_half(h):
                sl = slice(1024 * h, 1024 * (h + 1))
                nc.sync.dma_start(kq2[0:KQ, sl], kqt[64 : 64 + KQ, sl])
                nc.sync.dma_start(kq2[64 : 64 + KQ, sl], kqt[0:KQ, sl])

            pvs = [None]

            def v_mm(part):
                if part == 0:
                    pvs[0] = pw.tile([128, 1024], FP32, name="pwm")
                pv = pvs[0]
                for jj in range(8 * part, 8 * part + 8):
                    nc.tensor.matmul(
                        pv[:, ts(jj, C)], xp[:, ts(jj, 128)], wv[:],
                        start=True, stop=True,
                    )

            def v_tanh():
                nc.scalar.activation(vsb[:], pvs[0][:], AF.Tanh)

            # first 10 tasks are the critical chain to the first exp of the
            # batch; the rest can lag
            if b == 0:
                # batch 0 runs fully upfront: split qdup per l-half so the
                # first mm2/exp only waits on elu(h0), not both halves
                tasks = [
                    lambda: kq_mm(0),
                    lambda: elu(0),
                    lambda: qdup_half(0),
                    lambda: kq_mm(1),
                    lambda: elu(1),
                    lambda: qdup_half(1),
                    lambda: v_mm(0),
                    lambda: v_mm(1),
                    v_tanh,
                ]
            else:
                tasks = []
                for h in range(2):
                    tasks.append(lambda h=h: kq_mm(h))
                    tasks.append(lambda h=h: elu(h))
                tasks.append(qdup)
                tasks.append(lambda: v_mm(0))
                tasks.append(lambda: v_mm(1))
                tasks.append(v_tanh)
            return kqt, kq2, vsb, tasks

        # PE warmup: ~4.5us of back-to-back dummy matmuls during the DMA
        # prologue (PE is otherwise idle) to engage the HAM 8/8 clock
        wz = const.tile([128, 512], BF16)
        nc.vector.memset(wz[:], 0.0)
        # warm the DVE cache-reduce uop path (first use gives a bad sum)
        wd = const.tile([128, 1], FP32)
        nc.vector.tensor_scalar(
            wz[:, 0:64], wz[:, 0:64], 1.0, 0.0, OP.mult, OP.add,
            accum_out=wd[:],
        )
        pwarm = pw.tile([128, 1024], FP32, name="pwm")
        for r in range(10):
            nc.tensor.matmul(
                pwarm[:, 0:512], wz[:, 0:128], wz[:],
                start=True, stop=True,
            )

        kqt, kq2, vsb, tasks0 = prep_tasks(0)
        for t in tasks0[:6]:
            t()

        for b in range(BPC):
            if b + 1 < BPC:
                kqt_n, kq2_n, vsb_n, tasks = prep_tasks(b + 1)
            else:
                tasks = []

            pyt = py.tile([128, 1024], FP32, name="pyt")

            def emit_mm2(m):
                # Two concurrent row-tiled streams: A on PE rows 0-49
                # (l-half 0), B on rows 64-113 (l-half 1).
                tiles = [
                    pw.tile([128, 1024], FP32, name="pwm"),
                    pw.tile([128, 1024], FP32, name="pwm"),
                ]
                # B before A in each pair: tiles[1]'s PSUM buffer is freed
                # by the previous chunk's h1 exp, which (with h1-first exp
                # order below) completes before h0's, so stream B can start
                # while A still waits on its buffer
                for jj in range(2):
                    nc.tensor.matmul(
                        tiles[1][:, ts(jj, 512)],
                        kqt[64 : 64 + KQ, ts(m, 128)],
                        kq2[64 : 64 + KQ, ts(2 + jj, 512)],
                        start=True,
                        stop=True,
                    )
                    nc.tensor.matmul(
                        tiles[0][:, ts(jj, 512)],
                        kq2[0:KQ, ts(m, 128)],
                        kqt[0:KQ, ts(jj, 512)],
                        start=True,
                        stop=True,
                    )
                return tiles

            pw2 = emit_mm2(0)
            if b == 0:
                # batch-0 v/tanh after mm2(0): keeps the first exps off the
                # back of 16 v-matmuls on the PE queue
                for t in tasks0[6:]:
                    t()
            ti = 0

            def emit_mm3(m, vp, et):
                # y^T accum: quadrant (j//2) selects partition half; order
                # 0,2,1,3 pairs different col-groups for concurrency
                for j in (0, 2, 1, 3):
                    nc.tensor.matmul(
                        pyt[64 * (j // 2) : 64 * (j // 2) + C, ts(j % 2, 512)],
                        vp[:],
                        et[:, ts(j, 512)],
                        start=(m == 0),
                        stop=(m == NM - 1),
                    )

            for m in range(NM):
                et = epool.tile([128, L], BF16)
                d2 = spool.tile([128, 2], FP32)
                # h1 first: its ACT op carries no accumulator (the row-sum
                # comes from a DVE reduce over the bf16 et instead, halving
                # the costly ACTIVATION_READ_ACCUMULATOR count on ACT)
                nc.scalar.activation(
                    et[:, ts(1, 1024)], pw2[1][:], AF.Exp, scale=scale,
                )
                nc.scalar.activation(
                    et[:, ts(0, 1024)], pw2[0][:], AF.Exp, scale=scale,
                    accum_out=d2[:, 0:1],
                )
                nc.vector.tensor_reduce(
                    d2[:, 1:2], et[:, ts(1, 1024)], AX.X, OP.add
                )
                if m + 1 < NM:
                    pw2 = emit_mm2(m + 1)
                dsum = spool.tile([128, 1], FP32)
                nc.vector.tensor_add(dsum[:], d2[:, 0:1], d2[:, 1:2])
                dinv = spool.tile([128, 1], FP32)
                nc.vector.reciprocal(dinv[:], dsum[:])
                vp = vppool.tile([128, C], BF16)
                nc.vector.tensor_scalar_mul(vp[:], vsb[:, ts(m, C)], dinv[:])
                emit_mm3(m, vp, et)
                if ti < len(tasks) and m >= 2:
                    tasks[ti]()
                    ti += 1

            while ti < len(tasks):
                tasks[ti]()
                ti += 1

            yt = ytpool.tile([128, 1024], FP32)
            nc.vector.tensor_copy(yt[:], pyt[:])
            nc.sync.dma_start(y_d[b][0], yt[0:C, :])
            nc.sync.dma_start(y_d[b][1], yt[64 : 64 + C, :])

            if b + 1 < BPC:
                kqt, kq2, vsb = kqt_n, kq2_n, vsb_n

    nc.finalize()
    return nc


def kernel(x, Wk, bk, Wq, bq, Wv, bv, sample_len):
    global LAST_EXEC_NS
    from concourse.bass_utils import run_bass_kernel_spmd

    scale = float(1.0 / np.sqrt(np.float64(sample_len)))
    if scale not in _cache:
        _cache[scale] = _build(scale)
    nc = _cache[scale]

    import ml_dtypes

    bf16 = ml_dtypes.bfloat16
    x = np.asarray(x, dtype=np.float32)
    ones = np.ones((B, 1, L), dtype=np.float32)
    x = np.ascontiguousarray(np.concatenate([x, ones], axis=1)).astype(bf16)
    wkq = np.zeros((C + 1, 128), dtype=np.float32)
    wkq[:, 0:KQ] = np.concatenate([Wk, bk[None, :]], axis=0)
    wkq[:, 64 : 64 + KQ] = np.concatenate([Wq, bq[None, :]], axis=0)
    wkq = wkq.astype(bf16)
    wv = np.concatenate([Wv, bv[None, :]], axis=0).astype(bf16)

    in_maps = [
        {"x": x[i * BPC : (i + 1) * BPC], "wkq": wkq, "wv": wv}
        for i in range(NCORES)
    ]
    res = run_bass_kernel_spmd(nc, in_maps, list(range(NCORES)), trace=TRACE)
    LAST_EXEC_NS = res.exec_time_ns
    yp = np.concatenate([res.results[i]["y"] for i in range(NCORES)], axis=0)
    # yp: [B, 2, 64, 1024] -> y: [B, 2048, 64]
    y = yp.transpose(0, 1, 3, 2).reshape(B, L, C)
    return np.ascontiguousarray(y)
